# revision 7
# baseline (speedup 1.0000x reference)
"""BSpline activation kernel for 8 TRN2 NeuronCores.

f(x) = sum_i c_i * B_i(clip(x, -1, 1)) with cubic B-splines over a uniform
12-knot grid.  f is piecewise cubic with C2 continuity, so it can be written
in truncated-power form:

    f(xc) = P_0(xc) + sum_{j=1}^{10} d_j * relu(xc - g_j)^3

where P_0 is the cubic on the leftmost interval and d_j are the x^3
coefficient jumps at the interior knots.  The baseline kernel evaluates this
with ScalarE (relu, square) + VectorE fused multiply-adds, data-parallel
over 8 cores (each core gets a 256x4096 shard).
"""

import os
import sys

import numpy as np

sys.path.insert(0, "/opt/trn_rl_repo")

SPLINE_ORDER = 3
N_KNOTS = 12
N_BASES = 8  # N_KNOTS - 1 - SPLINE_ORDER
IN_LO, IN_HI = -1.0, 1.0
DENOM_EPS = 1e-8

FULL_SHAPE = (2048, 4096)
N_CORES = 8
ROWS_PER_CORE = FULL_SHAPE[0] // N_CORES  # 256
# Per-core shard viewed as [128, FREE] for SBUF layout
P = 128
FREE = ROWS_PER_CORE * FULL_SHAPE[1] // P  # 8192
TILE_F = 2048
N_TILES = FREE // TILE_F


def _bspline_bases_np(x, grid):
    """Cox-de Boor, float64, mirrors reference.py exactly."""
    xf = x[..., None]
    B = ((grid[:-1] <= xf) & (xf < grid[1:])).astype(np.float64)
    for k in range(1, SPLINE_ORDER + 1):
        g_i = grid[: -(k + 1)]
        g_ik = grid[k:-1]
        g_i1 = grid[1:-k]
        g_ik1 = grid[k + 1:]
        d1 = g_ik - g_i
        d2 = g_ik1 - g_i1
        w1 = np.where(d1 > DENOM_EPS, (xf - g_i) / np.where(d1 > DENOM_EPS, d1, 1.0), 0.0)
        w2 = np.where(d2 > DENOM_EPS, (g_ik1 - xf) / np.where(d2 > DENOM_EPS, d2, 1.0), 0.0)
        B = w1 * B[..., :-1] + w2 * B[..., 1:]
    return B


def interval_polys(grid, coefficients):
    """Exact cubic power-basis coefficients of f on each knot interval.

    Returns polys[j] = [a0, a1, a2, a3] such that
    f(x) = a0 + a1 x + a2 x^2 + a3 x^3 on [g_j, g_{j+1}), j = 0..10.
    """
    grid = np.asarray(grid, dtype=np.float64)
    coefficients = np.asarray(coefficients, dtype=np.float64)
    polys = []
    for j in range(N_KNOTS - 1):
        lo, hi = grid[j], grid[j + 1]
        # 4 interior sample points (avoid the half-open right edge)
        ts = lo + (hi - lo) * np.array([0.125, 0.375, 0.625, 0.875])
        B = _bspline_bases_np(ts, grid)  # [4, 8]
        fv = B @ coefficients
        V = np.vander(ts, 4, increasing=True)  # [1, t, t^2, t^3]
        polys.append(np.linalg.solve(V, fv))
    return np.array(polys)  # [11, 4]


def truncated_power_form(grid, polys):
    """P_0 coefficients and d_j (j=1..10) truncated-power coefficients."""
    p0 = polys[0]
    djs = polys[1:, 3] - polys[:-1, 3]  # x^3 coefficient jumps
    return p0, djs


def spline_eval_host(x, grid, polys):
    """Pure-numpy evaluation used for self-checks."""
    g = np.asarray(grid, np.float64)
    xc = np.clip(x.astype(np.float64), IN_LO, IN_HI)
    idx = np.clip(np.searchsorted(g, xc, side="right") - 1, 0, N_KNOTS - 2)
    a = polys[idx]  # [..., 4]
    return a[..., 0] + xc * (a[..., 1] + xc * (a[..., 2] + xc * a[..., 3]))


# ---------------------------------------------------------------------------
# Bass kernel
# ---------------------------------------------------------------------------

_cache = {}


def _build_nc(grid, coefficients):
    import concourse.bacc as bacc
    import concourse.mybir as mybir
    import concourse.tile as tile

    polys = interval_polys(grid, coefficients)
    p0, djs = truncated_power_form(np.asarray(grid, np.float64), polys)
    knots = np.asarray(grid, np.float64)[1:11]  # g_1..g_10

    nc = bacc.Bacc("TRN2", target_bir_lowering=False, num_devices=N_CORES)
    dt = mybir.dt.float32
    x_ext = nc.declare_dram_parameter("x", [P, FREE], dt, isOutput=False)
    out_ext = nc.declare_dram_parameter("out", [P, FREE], dt, isOutput=True)

    Alu = mybir.AluOpType
    Act = mybir.ActivationFunctionType

    with tile.TileContext(nc) as tc:
        with tc.tile_pool(name="consts", bufs=1) as cpool, \
             tc.tile_pool(name="pool", bufs=3) as pool:
            bias_t = cpool.tile([P, 10], dt, tag="bias")
            for j in range(10):
                nc.vector.memset(bias_t[:, j : j + 1], float(-knots[j]))
            for i in range(N_TILES):
                sl = slice(i * TILE_F, (i + 1) * TILE_F)
                xt = pool.tile([P, TILE_F], dt, tag="xt")
                nc.sync.dma_start(out=xt[:], in_=x_ext[:, sl])
                xc = pool.tile([P, TILE_F], dt, tag="xc")
                nc.vector.tensor_scalar(
                    xc[:], xt[:], float(IN_LO), float(IN_HI), Alu.max, Alu.min
                )
                # Horner for P_0: acc = a3*x + a2; acc = acc*x + a1; acc = acc*x + a0
                acc = pool.tile([P, TILE_F], dt, tag="acc")
                nc.vector.tensor_scalar(
                    acc[:], xc[:], float(p0[3]), float(p0[2]), Alu.mult, Alu.add
                )
                tmp = pool.tile([P, TILE_F], dt, tag="tmp")
                nc.vector.scalar_tensor_tensor(
                    tmp[:], acc[:], 1.0, xc[:], Alu.mult, Alu.mult
                )
                nc.vector.tensor_scalar(
                    acc[:], tmp[:], float(p0[1]), None, Alu.add
                )
                nc.vector.scalar_tensor_tensor(
                    tmp[:], acc[:], 1.0, xc[:], Alu.mult, Alu.mult
                )
                nc.vector.tensor_scalar(
                    acc[:], tmp[:], float(p0[0]), None, Alu.add
                )
                r = pool.tile([P, TILE_F], dt, tag="r")
                r2 = pool.tile([P, TILE_F], dt, tag="r2")
                for j in range(10):
                    nc.scalar.activation(
                        r[:], xc[:], Act.Relu, bias=bias_t[:, j : j + 1], scale=1.0
                    )
                    nc.scalar.activation(r2[:], r[:], Act.Square)
                    # tmp = (r2 * d_j) * r
                    nc.vector.scalar_tensor_tensor(
                        tmp[:], r2[:], float(djs[j]), r[:], Alu.mult, Alu.mult
                    )
                    nc.vector.tensor_tensor(
                        out=acc[:], in0=acc[:], in1=tmp[:], op=Alu.add
                    )
                nc.sync.dma_start(out=out_ext[:, sl], in_=acc[:])
    nc.finalize()
    return nc


def _build_nc_table(marker, reps=1):
    """Table mode: the whole spline is baked into the ScalarE activation
    table (replacing `exp`), so the kernel is DMA-in -> ACTIVATE -> DMA-out.
    `marker` is a table-content hash memset into a dummy tile purely to make
    the BIR (and thus the NEFF cache key) unique per table contents."""
    import concourse.bacc as bacc
    import concourse.mybir as mybir
    import concourse.tile as tile

    nc = bacc.Bacc("TRN2", target_bir_lowering=False, num_devices=N_CORES)
    dt = mybir.dt.float32
    x_ext = nc.declare_dram_parameter("x", [P, FREE], dt, isOutput=False)
    out_ext = nc.declare_dram_parameter("out", [P, FREE], dt, isOutput=True)
    Act = mybir.ActivationFunctionType

    with tile.TileContext(nc) as tc:
        with tc.tile_pool(name="consts", bufs=1) as cpool, \
             tc.tile_pool(name="pool", bufs=4) as pool:
            mark = cpool.tile([P, 1], dt, tag="marker")
            nc.vector.memset(mark[:], float(marker))
            for _rep in range(reps):
              for i in range(N_TILES):
                sl = slice(i * TILE_F, (i + 1) * TILE_F)
                xt = pool.tile([P, TILE_F], dt, tag="xt")
                nc.sync.dma_start(out=xt[:], in_=x_ext[:, sl])
                yt = pool.tile([P, TILE_F], dt, tag="yt")
                nc.scalar.activation(yt[:], xt[:], Act.Exp, bias=0.0, scale=1.0)
                # out-DMAs on gpsimd: keeps each engine's instruction stream
                # short (IRAM block) and lets in/out queues run independently
                nc.gpsimd.dma_start(out=out_ext[:, sl], in_=yt[:])
    nc.finalize()
    return nc


def kernel(x, grid, coefficients):
    from concourse.bass_utils import run_bass_kernel_spmd

    mode = os.environ.get("KERNEL_MODE", "table")
    if mode == "table":
        import hashlib
        import tempfile

        import acttab

        key = ("table", grid.tobytes(), coefficients.tobytes())
        if key not in _cache:
            polys = interval_polys(grid, coefficients)
            out_dir = tempfile.mkdtemp(prefix="actroot_")
            act_root = acttab.build_act_root(grid, polys, out_dir)
            h = hashlib.sha256()
            for fn in ("exp_and_others_bkt.bin", "exp_and_others_ctrl.bin",
                       "exp_and_others.json"):
                h.update(open(os.path.join(out_dir, fn), "rb").read())
            marker = int.from_bytes(h.digest()[:3], "little")  # 24-bit
            os.environ["BASS_ACT_ROOT_JSON_PATH"] = act_root
            _cache[key] = (_build_nc_table(marker), act_root)
        nc, act_root = _cache[key]
        os.environ["BASS_ACT_ROOT_JSON_PATH"] = act_root
    else:
        key = (grid.tobytes(), coefficients.tobytes())
        if key not in _cache:
            _cache[key] = _build_nc(grid, coefficients)
        nc = _cache[key]

    x = np.ascontiguousarray(x, dtype=np.float32)
    shards = x.reshape(N_CORES, P, FREE)
    in_maps = [{"x": shards[i]} for i in range(N_CORES)]
    res = run_bass_kernel_spmd(
        nc, in_maps, core_ids=list(range(N_CORES)), trace=False
    )
    kernel._last_results = res
    out = np.stack([r["out"] for r in res.results])  # [8, 128, FREE]
    return out.reshape(FULL_SHAPE)


kernel._last_results = None
